# revision 8
# baseline (speedup 1.0000x reference)
"""Trainium2 Bass kernel for BondEncoding2D (Graphormer-style bond encoding).

Computes, for a 512x512 node-pair grid:
  phi_spd[h,i,j]  = spd_table[spatial_pos[i,j], h]
  phi_edge[h,i,j] = (sum_d edge_table[edge_input[i,j,d]] @ W[d])[h] / max(spatial_pos[i,j],1)

Sharding: rows of the grid across 8 NeuronCores (64 rows / 32768 pairs each);
tables and weights replicated (per the sharding hint).

Per-core strategy (v4 - moving one-hot, all-fp8 DoubleRow):
  * Host precomputes M[d] = edge_table @ W[d]; the edge term is then
    edge_sum[pair,:] = onehot.T @ Mflat over combos c=(d,bond) [512 dims].
  * The one-hot is built on host as exact fp8 and kept RESIDENT in SBUF.
    It is the MOVING matmul operand (N=512 pairs per matmul); the tiny M
    matrices are stationary -- the PE streams at full rate instead of
    reloading a 128x128 stationary every 32 columns (the old baseline).
  * DoubleRow packs 2 fp8 contraction rows per PE cell (K=256/pass, 2x):
    edge = 4 DR matmuls per 512-pair tile ({hi,lo} x 2 combo halves),
    hi/lo split of M accumulated in PSUM (no DVE add needed).
  * spd gather AND the 1/denom broadcast come from ONE extra DR matmul
    against the spatial one-hot: stationary cols 0-31 = 1/max(s,1) (hi/lo
    fp8), cols 32-63 = spd_table (hi/lo fp8); the moving one-hot is
    Ko-broadcast (step-0 AP).  Output SR[64,512]: R rows 0-31, S rows 32-63.
  * Epilogue: DVE multiplies E * R -> bf16 staging (rows 0-31); ACT copies
    S -> bf16 staging (rows 32-63).  Outputs upcast on host.
"""

import numpy as np
import ml_dtypes

import concourse.bass as bass
import concourse.bacc as bacc
import concourse.mybir as mybir
import concourse.tile as tile
from concourse.bass_utils import run_bass_kernel_spmd

N = 512          # atoms
D = 16           # max_dist
H = 32           # heads
NS = 64          # spatial values
NCORES = 8
RC = N // NCORES          # rows per core (64)
PC = RC * N               # pairs per core (32768)

TILES = 64                # tiles per core (one grid row each, 512 pairs)
TP = 512                  # pairs per tile
GT = 4                    # tiles per reorder group (stationary amortization)
DMA_T = 8                 # tiles batched per output DMA

BF16 = mybir.dt.bfloat16
F32 = mybir.dt.float32
FP8 = mybir.dt.float8e4
NP8 = ml_dtypes.float8_e4m3
NBF16 = ml_dtypes.bfloat16

_cached = {}


def _build_nc(bench_reps=None, parts=("edge_mm", "sr_mm", "epi", "dma")):
    flags = set(parts)
    nc = bacc.Bacc(None, target_bir_lowering=False)

    DR = mybir.MatmulPerfMode.DoubleRow

    # edge one-hot: [ki, (t, mm, ko, j)]; combo c = mm*256 + ko*128 + ki
    ohe = nc.dram_tensor("ohe", [128, TILES * 2048], FP8, kind="ExternalInput")
    # spatial one-hot [s, pair]
    ohs = nc.dram_tensor("ohs", [NS, PC], FP8, kind="ExternalInput")
    # edge stationaries: [ki, (w, ko, h)], w = 2*level + mm
    wts = nc.dram_tensor("wts", [128, 256], FP8, kind="ExternalInput")
    # spd/rdev stationary: [s, (ko, m)]; ko = hi/lo level,
    # m 0-31 = 1/max(s,1) replicated, m 32-63 = spd_table
    wsr = nc.dram_tensor("wsr", [NS, 128], FP8, kind="ExternalInput")
    oedge = nc.dram_tensor("oedge", [H, TILES * TP], BF16,
                           kind="ExternalOutput")
    ospd = nc.dram_tensor("ospd", [H, TILES * TP], BF16,
                          kind="ExternalOutput")

    mult = mybir.AluOpType.mult

    with tile.TileContext(nc) as tc:
        with (
            tc.tile_pool(name="consts", bufs=1) as cpool,
            tc.tile_pool(name="pe_e", bufs=4, space="PSUM") as epool,
            tc.tile_pool(name="pe_sr", bufs=4, space="PSUM") as srpool,
            tc.tile_pool(name="stage", bufs=2) as stpool,
        ):
            ohe_t = cpool.tile([128, TILES * 2048], FP8)
            nc.sync.dma_start(ohe_t[:], ohe[:])
            ohs_t = cpool.tile([NS, PC], FP8)
            nc.sync.dma_start(ohs_t[:], ohs[:])
            wts_t = cpool.tile([128, 256], FP8)
            nc.sync.dma_start(wts_t[:], wts[:])
            wsr_t = cpool.tile([NS, 128], FP8)
            nc.sync.dma_start(wsr_t[:], wsr[:])

            import contextlib
            loop_cm = (
                tc.For_i(0, bench_reps, 1) if bench_reps
                else contextlib.nullcontext()
            )
            with loop_cm:
                for g in range(TILES // GT):
                    if (g * GT) % DMA_T == 0:
                        sb = stpool.tile([2 * H, DMA_T * TP], BF16, tag="sb")
                    E = [epool.tile([H, TP], F32, tag="E", name=f"E{g}_{k}")
                         for k in range(GT)]
                    SR = [srpool.tile([2 * H, TP], F32, tag="SR",
                                      name=f"SR{g}_{k}")
                          for k in range(GT)]
                    if "edge_mm" in flags:
                        for w in range(4):
                            lhsT = wts_t[:, w * 64:w * 64 + 64].rearrange(
                                "p (ko h) -> p ko h", ko=2)
                            mm = w & 1
                            for k in range(GT):
                                t = GT * g + k
                                rb = t * 2048 + mm * 1024
                                rhs = ohe_t[:, rb:rb + 1024].rearrange(
                                    "p (ko j) -> p ko j", ko=2)
                                nc.tensor.matmul(
                                    E[k][:], lhsT, rhs,
                                    start=(w == 0), stop=(w == 3),
                                    perf_mode=DR,
                                )
                    if "sr_mm" in flags:
                        lhsT = wsr_t[:].rearrange("p (ko m) -> p ko m", ko=2)
                        for k in range(GT):
                            t = GT * g + k
                            mv = ohs_t[:, t * TP:(t + 1) * TP].rearrange(
                                "p (o j) -> p o j", o=1
                            ).broadcast_to((NS, 2, TP))
                            nc.tensor.matmul(
                                SR[k][:], lhsT, mv,
                                start=True, stop=True, perf_mode=DR,
                            )
                    if "epi" in flags:
                        for k in range(GT):
                            t = GT * g + k
                            col = (t % DMA_T) * TP
                            # one ACT op evacuates R (rows 0-31) and S
                            # (rows 32-63) to bf16 staging; DVE then
                            # overwrites rows 0-31 in place with E * R.
                            nc.scalar.copy(
                                sb[0:2 * H, col:col + TP], SR[k][:])
                            nc.vector.tensor_tensor(
                                sb[0:H, col:col + TP],
                                E[k][:], sb[0:H, col:col + TP], mult)
                    if "dma" in flags:
                        if (GT * g + GT) % DMA_T == 0:
                            b = (GT * g + GT) // DMA_T - 1
                            sl = slice(b * DMA_T * TP, (b + 1) * DMA_T * TP)
                            nc.sync.dma_start(oedge[:, sl], sb[0:H, :])
                            nc.sync.dma_start(ospd[:, sl], sb[H:2 * H, :])
    nc.compile()
    return nc


def _host_prep(spatial_pos, edge_input, max_dist, spd_table, edge_table,
               edge_dis_weight):
    """Build per-core input maps (all numpy)."""
    md = int(max_dist)
    assert md == D
    W = edge_dis_weight.reshape(-1, H, H)[:md].astype(np.float64)
    M = edge_table.astype(np.float64) @ W          # (16, 32, 32)
    Mflat = M.reshape(512, H)                      # c=(d*32+b) -> [c, h]
    Mhi = Mflat.astype(NP8)
    Mlo = (Mflat - Mhi.astype(np.float64)).astype(NP8)

    # wts[ki, w*64 + ko*32 + h] = Mlevel[mm*256 + ko*128 + ki, h], w=2*li+mm
    wts = np.zeros((128, 256), NP8)
    for li, Ml in enumerate((Mhi, Mlo)):
        for mm in range(2):
            w = 2 * li + mm
            for ko in range(2):
                c0 = mm * 256 + ko * 128
                wts[:, w * 64 + ko * 32:w * 64 + ko * 32 + 32] = \
                    Ml[c0:c0 + 128]

    # wsr[s, ko*64 + m]: m 0-31 = 1/max(s,1), m 32-63 = spd_table[s]
    sv = np.arange(NS)
    rv = (1.0 / np.maximum(sv, 1)).astype(np.float64)
    rhi = rv.astype(NP8)
    rlo = (rv - rhi.astype(np.float64)).astype(NP8)
    st64 = spd_table.astype(np.float64)            # (64, 32)
    shi = st64.astype(NP8)
    slo = (st64 - shi.astype(np.float64)).astype(NP8)
    wsr = np.zeros((NS, 128), NP8)
    wsr[:, 0:32] = rhi[:, None]
    wsr[:, 32:64] = shi
    wsr[:, 64:96] = rlo[:, None]
    wsr[:, 96:128] = slo

    cc = np.arange(512)
    dd = (cc >> 5).astype(np.int32)
    bb = (cc & 31).astype(np.int32)
    svals = np.arange(NS, dtype=np.int32)
    in_maps = []
    for c in range(NCORES):
        rows = slice(RC * c, RC * (c + 1))
        e = edge_input[rows].reshape(TILES, TP, D)           # (t, j, d)
        oh = (e[:, :, dd] == bb)                             # (t, j, c)
        # -> ohe[ki, t*2048 + mm*1024 + ko*512 + j]
        oh2 = oh.transpose(2, 0, 1).reshape(2, 2, 128, TILES, TP)
        ohe = np.ascontiguousarray(
            oh2.transpose(2, 3, 0, 1, 4).reshape(128, TILES * 2048)
        ).astype(NP8)
        sp = spatial_pos[rows].reshape(PC)
        ohs = np.ascontiguousarray(
            (sp[None, :] == svals[:, None])).astype(NP8)     # (64, PC)
        in_maps.append({"ohe": ohe, "ohs": ohs, "wts": wts, "wsr": wsr})
    return in_maps


def _host_assemble(results):
    phi_spd = np.empty((H, N, N), np.float32)
    phi_edge = np.empty((H, N, N), np.float32)
    for c in range(NCORES):
        rs = slice(RC * c, RC * (c + 1))
        for name, dst in (("ospd", phi_spd), ("oedge", phi_edge)):
            a = np.asarray(results[c][name]).astype(np.float32)
            dst[:, rs, :] = a.reshape(H, RC, N)
    return phi_spd, phi_edge


def kernel(spatial_pos, edge_input, max_dist, spd_table, edge_table,
           edge_dis_weight, _trace=False):
    spatial_pos = np.asarray(spatial_pos)
    edge_input = np.asarray(edge_input)
    spd_table = np.asarray(spd_table, dtype=np.float32)
    edge_table = np.asarray(edge_table, dtype=np.float32)
    edge_dis_weight = np.asarray(edge_dis_weight, dtype=np.float32)

    if "nc" not in _cached:
        _cached["nc"] = _build_nc()
    nc = _cached["nc"]

    in_maps = _host_prep(spatial_pos, edge_input, max_dist, spd_table,
                         edge_table, edge_dis_weight)
    res = run_bass_kernel_spmd(
        nc, in_maps, core_ids=list(range(NCORES)), trace=bool(_trace)
    )
    out = _host_assemble(res.results)
    if _trace:
        return out, res
    return out


# revision 9
# speedup vs baseline: 4.1498x; 4.1498x over previous
"""BondEncoding2D Trainium2 kernel (final: v6.5).

Like v6.1/6.2 (2 DR edge matmuls/tile + compacted lo correction + 2-tile
spd matmul + streamed 1/denom) but PSUM is organized as ONE [128,1024]
tile (2 banks) per 2-tile pair-group:
  * E of tile even -> [0:32, 0:512] (bank A), tile odd -> [0:32, 512:1024]
    (bank B);
  * S of both tiles -> [64:128, 0:512] (bank A, via tile_position (0,64)).
With bufs=4 that keeps 4 pair-groups (8 tiles) in flight on 8 banks, so the
PE can run well ahead of the DVE/ACT drains.
"""

import numpy as np
import ml_dtypes

import concourse.bass as bass
import concourse.bacc as bacc
import concourse.mybir as mybir
import concourse.tile as tile
from concourse.bass_utils import run_bass_kernel_spmd

N = 512
D = 16
H = 32
NS = 64
NCORES = 8
RC = N // NCORES
PC = RC * N

TILES = 64
TP = 512
PGS = TILES // 2          # 32 pair-groups
NLO = 4
LOSLOTS = NLO * TP

BF16 = mybir.dt.bfloat16
F32 = mybir.dt.float32
FP8 = mybir.dt.float8e4
NP8 = ml_dtypes.float8_e4m3
NBF16 = ml_dtypes.bfloat16

_cached = {}


def _build_nc(bench_reps=None, parts=("edge_mm", "s_mm", "lo", "epi", "dma")):
    flags = set(parts)
    nc = bacc.Bacc(None, target_bir_lowering=False)

    DR = mybir.MatmulPerfMode.DoubleRow

    ohe = nc.dram_tensor("ohe", [128, TILES * 2048], FP8, kind="ExternalInput")
    ohlo = nc.dram_tensor("ohlo", [128, NLO * 2048], FP8, kind="ExternalInput")
    ohs = nc.dram_tensor("ohs", [128, PC // 2], FP8, kind="ExternalInput")
    wts = nc.dram_tensor("wts", [128, 128], FP8, kind="ExternalInput")
    wlo = nc.dram_tensor("wlo", [128, 128], FP8, kind="ExternalInput")
    stab = nc.dram_tensor("stab", [128, 64], BF16, kind="ExternalInput")
    rdev = nc.dram_tensor("rdev", [1, PC], BF16, kind="ExternalInput")
    oedge = nc.dram_tensor("oedge", [H, PC], BF16, kind="ExternalOutput")
    ospd = nc.dram_tensor("ospd", [128, PC // 4], BF16, kind="ExternalOutput")
    olo = nc.dram_tensor("olo", [H, LOSLOTS], BF16, kind="ExternalOutput")

    mult = mybir.AluOpType.mult

    with tile.TileContext(nc) as tc:
        with (
            tc.tile_pool(name="consts", bufs=1) as cpool,
            tc.tile_pool(name="pe_p", bufs=3, space="PSUM") as ppool,
            tc.tile_pool(name="pe_s", bufs=2, space="PSUM") as spool,
            tc.tile_pool(name="st_e", bufs=3) as sepool,
            tc.tile_pool(name="st_s", bufs=2) as sspool,
            tc.tile_pool(name="st_r", bufs=8) as srpool,
            tc.tile_pool(name="st_lo", bufs=1) as slopool,
        ):
            ohe_t = cpool.tile([128, TILES * 2048], FP8)
            nc.sync.dma_start(ohe_t[:], ohe[:])
            ohlo_t = cpool.tile([128, NLO * 2048], FP8)
            nc.sync.dma_start(ohlo_t[:], ohlo[:])
            ohs_t = cpool.tile([128, PC // 2], FP8)
            nc.sync.dma_start(ohs_t[:], ohs[:])
            wts_t = cpool.tile([128, 128], FP8)
            nc.sync.dma_start(wts_t[:], wts[:])
            wlo_t = cpool.tile([128, 128], FP8)
            nc.sync.dma_start(wlo_t[:], wlo[:])
            stab_t = cpool.tile([128, 64], BF16)
            nc.sync.dma_start(stab_t[:], stab[:])

            import contextlib
            loop_cm = (
                tc.For_i(0, bench_reps, 1) if bench_reps
                else contextlib.nullcontext()
            )
            with loop_cm:
                rsts = {}

                def issue_rst(pg):
                    t = srpool.tile([H, 2 * TP], BF16, tag="rst",
                                    name=f"rst{pg}")
                    src = rdev[:, pg * 2 * TP:(pg + 1) * 2 * TP]
                    nc.sync.dma_start(t[:], src.broadcast_to((H, 2 * TP)))
                    rsts[pg] = t

                for pg in range(PGS):
                    if pg % 4 == 0:
                        sbe = sepool.tile([H, 8 * TP], BF16, tag="sbe",
                                          name=f"sbe{pg}")
                    if pg % 8 == 0:
                        sbs = sspool.tile([128, 4 * TP], BF16, tag="sbs",
                                          name=f"sbs{pg}")
                    if "epi" in flags:
                        if pg == 0:
                            for q in range(6):
                                issue_rst(q)
                        if pg + 6 < PGS:
                            issue_rst(pg + 6)
                        rst = rsts.pop(pg)
                    P = ppool.tile([128, 2 * TP], F32, tag="P",
                                   name=f"P{pg}")
                    if "edge_mm" in flags:
                        for w in range(2):
                            lhsT = wts_t[:, w * 64:w * 64 + 64].rearrange(
                                "p (ko h) -> p ko h", ko=2)
                            for k2 in range(2):
                                t = 2 * pg + k2
                                rb = t * 2048 + w * 1024
                                rhs = ohe_t[:, rb:rb + 1024].rearrange(
                                    "p (ko j) -> p ko j", ko=2)
                                nc.tensor.matmul(
                                    P[0:H, k2 * TP:(k2 + 1) * TP],
                                    lhsT, rhs,
                                    start=(w == 0), stop=(w == 1),
                                    perf_mode=DR,
                                )
                    if "s_mm" in flags:
                        if pg % 2 == 0:
                            S4 = spool.tile([128, TP], F32, tag="S4",
                                            name=f"S4{pg}")
                        nc.tensor.matmul(
                            S4[64 * (pg % 2):64 * (pg % 2) + 64, :],
                            stab_t[:], ohs_t[:, pg * TP:(pg + 1) * TP],
                            start=True, stop=True,
                            tile_position=(0, 64 * (pg % 2)),
                        )
                    if "epi" in flags:
                        col = (pg % 4) * 2 * TP
                        nc.vector.tensor_tensor(
                            sbe[:, col:col + 2 * TP], P[0:H, :], rst[:], mult)
                        if pg % 2 == 1:
                            q = pg // 2
                            nc.scalar.copy(
                                sbs[:, (q % 4) * TP:(q % 4 + 1) * TP], S4[:])
                    if "dma" in flags and pg % 4 == 3:
                        b = pg // 4
                        hw = 4 * TP
                        nc.sync.dma_start(
                            oedge[:, b * 8 * TP:b * 8 * TP + hw],
                            sbe[:, 0:hw])
                        nc.sync.dma_start(
                            oedge[:, b * 8 * TP + hw:(b + 1) * 8 * TP],
                            sbe[:, hw:2 * hw])
                    if "dma" in flags and pg % 8 == 7:
                        b2 = pg // 8
                        nc.sync.dma_start(
                            ospd[:, b2 * 4 * TP:(b2 + 1) * 4 * TP], sbs[:])
                    if "lo" in flags and pg % 16 == 15:
                        lt2 = pg // 16      # 0 or 1 -> lo tiles 2lt2, 2lt2+1
                        if lt2 == 0:
                            sblo = slopool.tile([H, LOSLOTS], BF16,
                                                tag="sblo", name=f"sblo{pg}")
                        Plo = ppool.tile([128, 2 * TP], F32, tag="P",
                                         name=f"Plo{lt2}")
                        for k2 in range(2):
                            lt = 2 * lt2 + k2
                            for w in range(2):
                                lhsT = wlo_t[:, w * 64:w * 64 + 64].rearrange(
                                    "p (ko h) -> p ko h", ko=2)
                                rb = lt * 2048 + w * 1024
                                rhs = ohlo_t[:, rb:rb + 1024].rearrange(
                                    "p (ko j) -> p ko j", ko=2)
                                nc.tensor.matmul(
                                    Plo[0:H, k2 * TP:(k2 + 1) * TP],
                                    lhsT, rhs,
                                    start=(w == 0), stop=(w == 1),
                                    perf_mode=DR)
                        nc.scalar.copy(
                            sblo[:, lt2 * 2 * TP:(lt2 + 1) * 2 * TP],
                            Plo[0:H, :])
                        if lt2 == 1:
                            nc.sync.dma_start(olo[:], sblo[:])
    nc.compile()
    return nc


def _host_prep(spatial_pos, edge_input, max_dist, spd_table, edge_table,
               edge_dis_weight):
    md = int(max_dist)
    assert md == D
    W = edge_dis_weight.reshape(-1, H, H)[:md].astype(np.float64)
    M = edge_table.astype(np.float64) @ W
    Mflat = M.reshape(512, H)
    Mhi = Mflat.astype(NP8)
    Mlo = (Mflat - Mhi.astype(np.float64)).astype(NP8)

    def pack_w(Ml):
        w = np.zeros((128, 128), NP8)
        for mm in range(2):
            for ko in range(2):
                c0 = mm * 256 + ko * 128
                w[:, mm * 64 + ko * 32:mm * 64 + ko * 32 + 32] = \
                    Ml[c0:c0 + 128]
        return w
    wts = pack_w(Mhi)
    wlo = pack_w(Mlo)

    stab = np.zeros((128, 64), NBF16)
    stab[0:64, 0:32] = spd_table.astype(NBF16)
    stab[64:128, 32:64] = spd_table.astype(NBF16)

    cc = np.arange(512)
    dd = (cc >> 5).astype(np.int32)
    bb = (cc & 31).astype(np.int32)
    svals = np.arange(NS, dtype=np.int32)

    def edge_onehot(e):
        T = e.shape[0]
        oh = (e[:, :, dd] == bb)
        oh2 = oh.transpose(2, 0, 1).reshape(2, 2, 128, T, TP)
        return np.ascontiguousarray(
            oh2.transpose(2, 3, 0, 1, 4).reshape(128, T * 2048)).astype(NP8)

    in_maps = []
    for c in range(NCORES):
        rows = slice(RC * c, RC * (c + 1))
        e = edge_input[rows].reshape(TILES, TP, D)
        ohe = edge_onehot(e)
        sp = spatial_pos[rows]
        spf = sp.reshape(PC)
        oh1 = (spf[None, :] == svals[:, None])
        oh1 = oh1.reshape(NS, PGS, 2, TP)
        ohs = np.zeros((2, NS, PGS, TP), np.bool_)
        ohs[0] = oh1[:, :, 0, :]
        ohs[1] = oh1[:, :, 1, :]
        ohs = np.ascontiguousarray(ohs.reshape(128, PC // 2)).astype(NP8)
        rdev = np.ascontiguousarray(
            (1.0 / np.maximum(spf, 1)).astype(NBF16)[None, :])
        mask = (spf <= 2)
        idxs = np.nonzero(mask)[0]
        assert len(idxs) <= LOSLOTS, f"lo overflow: {len(idxs)}"
        elo = np.zeros((LOSLOTS, D), np.int32)
        elo[:len(idxs)] = e.reshape(PC, D)[idxs]
        ohlo = edge_onehot(elo.reshape(NLO, TP, D))
        in_maps.append({
            "ohe": ohe, "ohlo": ohlo, "ohs": ohs, "wts": wts, "wlo": wlo,
            "stab": stab, "rdev": rdev,
        })
    return in_maps


def _lo_idxs(spatial_pos, c):
    rows = slice(RC * c, RC * (c + 1))
    spf = spatial_pos[rows].reshape(PC)
    return np.nonzero(spf <= 2)[0]


def _host_assemble(results, spatial_pos):
    phi_spd = np.empty((H, N, N), np.float32)
    phi_edge = np.empty((H, N, N), np.float32)
    for c in range(NCORES):
        rs = slice(RC * c, RC * (c + 1))
        a = np.asarray(results[c]["ospd"]).astype(np.float32)
        # [32k+h, q*512+j], tile = 4*q + k (k = (pg%2)*2 + parity)
        a = a.reshape(4, H, TILES // 4, TP).transpose(1, 2, 0, 3)
        phi_spd[:, rs, :] = a.reshape(H, RC, N)
        b = np.asarray(results[c]["oedge"]).astype(np.float32)
        phi_edge[:, rs, :] = b.reshape(H, RC, N)
        idxs = _lo_idxs(spatial_pos, c)
        if len(idxs):
            lo = np.asarray(results[c]["olo"]).astype(np.float32)
            spf = spatial_pos[rs].reshape(PC).astype(np.float32)
            r = 1.0 / np.maximum(spf[idxs], 1)
            flat = phi_edge[:, rs, :].reshape(H, PC)
            flat[:, idxs] += lo[:, :len(idxs)] * r[None, :]
            phi_edge[:, rs, :] = flat.reshape(H, RC, N)
    return phi_spd, phi_edge


def kernel(spatial_pos, edge_input, max_dist, spd_table, edge_table,
           edge_dis_weight, _trace=False):
    spatial_pos = np.asarray(spatial_pos)
    edge_input = np.asarray(edge_input)
    spd_table = np.asarray(spd_table, dtype=np.float32)
    edge_table = np.asarray(edge_table, dtype=np.float32)
    edge_dis_weight = np.asarray(edge_dis_weight, dtype=np.float32)

    if "nc" not in _cached:
        _cached["nc"] = _build_nc()
    nc = _cached["nc"]

    in_maps = _host_prep(spatial_pos, edge_input, max_dist, spd_table,
                         edge_table, edge_dis_weight)
    res = run_bass_kernel_spmd(
        nc, in_maps, core_ids=list(range(NCORES)), trace=bool(_trace)
    )
    out = _host_assemble(res.results, spatial_pos)
    if _trace:
        return out, res
    return out


# revision 10
# speedup vs baseline: 4.5494x; 1.0963x over previous
"""BondEncoding2D Trainium2 kernel (final: v6.11).

Like v6.1/6.2 (2 DR edge matmuls/tile + compacted lo correction + 2-tile
spd matmul + streamed 1/denom) but PSUM is organized as ONE [128,1024]
tile (2 banks) per 2-tile pair-group:
  * E of tile even -> [0:32, 0:512] (bank A), tile odd -> [0:32, 512:1024]
    (bank B);
  * S of both tiles -> [64:128, 0:512] (bank A, via tile_position (0,64)).
With bufs=4 that keeps 4 pair-groups (8 tiles) in flight on 8 banks, so the
PE can run well ahead of the DVE/ACT drains.
"""

import numpy as np
import ml_dtypes

import concourse.bass as bass
import concourse.bacc as bacc
import concourse.mybir as mybir
import concourse.tile as tile
from concourse.bass_utils import run_bass_kernel_spmd

N = 512
D = 16
H = 32
NS = 64
NCORES = 8
RC = N // NCORES
PC = RC * N

TILES = 64
TP = 512
PGS = TILES // 2          # 32 pair-groups
NLO = 4
LOSLOTS = NLO * TP

BF16 = mybir.dt.bfloat16
F32 = mybir.dt.float32
FP8 = mybir.dt.float8e4
NP8 = ml_dtypes.float8_e4m3
NBF16 = ml_dtypes.bfloat16

_cached = {}


def _build_nc(bench_reps=None, parts=("edge_mm", "s_mm", "lo", "epi", "dma")):
    flags = set(parts)
    nc = bacc.Bacc(None, target_bir_lowering=False)

    DR = mybir.MatmulPerfMode.DoubleRow

    ohe = nc.dram_tensor("ohe", [128, TILES * 2048], FP8, kind="ExternalInput")
    ohlo = nc.dram_tensor("ohlo", [128, NLO * 2048], FP8, kind="ExternalInput")
    ohs = nc.dram_tensor("ohs", [128, PC // 2], FP8, kind="ExternalInput")
    wts = nc.dram_tensor("wts", [128, 128], FP8, kind="ExternalInput")
    wlo = nc.dram_tensor("wlo", [128, 128], FP8, kind="ExternalInput")
    stab = nc.dram_tensor("stab", [128, 64], BF16, kind="ExternalInput")
    rdev = nc.dram_tensor("rdev", [1, PC], BF16, kind="ExternalInput")
    oedge = nc.dram_tensor("oedge", [H, PC], BF16, kind="ExternalOutput")
    ospd = nc.dram_tensor("ospd", [128, PC // 4], BF16, kind="ExternalOutput")
    olo = nc.dram_tensor("olo", [H, LOSLOTS], BF16, kind="ExternalOutput")

    mult = mybir.AluOpType.mult

    with tile.TileContext(nc) as tc:
        with (
            tc.tile_pool(name="consts", bufs=1) as cpool,
            tc.tile_pool(name="pe_p", bufs=3, space="PSUM") as ppool,
            tc.tile_pool(name="pe_s", bufs=2, space="PSUM") as spool,
            tc.tile_pool(name="st_e", bufs=3) as sepool,
            tc.tile_pool(name="st_s", bufs=2) as sspool,
            tc.tile_pool(name="st_r", bufs=8) as srpool,
            tc.tile_pool(name="st_lo", bufs=1) as slopool,
        ):
            ohe_t = cpool.tile([128, TILES * 2048], FP8)
            nc.sync.dma_start(ohe_t[:], ohe[:])
            ohlo_t = cpool.tile([128, NLO * 2048], FP8)
            nc.sync.dma_start(ohlo_t[:], ohlo[:])
            ohs_t = cpool.tile([128, PC // 2], FP8)
            nc.sync.dma_start(ohs_t[:], ohs[:])
            wts_t = cpool.tile([128, 128], FP8)
            nc.sync.dma_start(wts_t[:], wts[:])
            wlo_t = cpool.tile([128, 128], FP8)
            nc.sync.dma_start(wlo_t[:], wlo[:])
            stab_t = cpool.tile([128, 64], BF16)
            nc.sync.dma_start(stab_t[:], stab[:])

            import contextlib
            loop_cm = (
                tc.For_i(0, bench_reps, 1) if bench_reps
                else contextlib.nullcontext()
            )
            with loop_cm:
                rsts = {}

                def issue_rst(pg):
                    t = srpool.tile([H, 2 * TP], BF16, tag="rst",
                                    name=f"rst{pg}")
                    src = rdev[:, pg * 2 * TP:(pg + 1) * 2 * TP]
                    nc.scalar.dma_start(t[:], src.broadcast_to((H, 2 * TP)))
                    rsts[pg] = t

                for pg in range(PGS):
                    if pg % 4 == 0:
                        sbe = sepool.tile([H, 8 * TP], BF16, tag="sbe",
                                          name=f"sbe{pg}")
                    if pg % 8 == 0:
                        sbs = sspool.tile([128, 4 * TP], BF16, tag="sbs",
                                          name=f"sbs{pg}")
                    if "epi" in flags:
                        if pg == 0:
                            for q in range(6):
                                issue_rst(q)
                        if pg + 6 < PGS:
                            issue_rst(pg + 6)
                        rst = rsts.pop(pg)
                    P = ppool.tile([128, 2 * TP], F32, tag="P",
                                   name=f"P{pg}")
                    if "edge_mm" in flags:
                        for w in range(2):
                            lhsT = wts_t[:, w * 64:w * 64 + 64].rearrange(
                                "p (ko h) -> p ko h", ko=2)
                            for k2 in range(2):
                                t = 2 * pg + k2
                                rb = t * 2048 + w * 1024
                                rhs = ohe_t[:, rb:rb + 1024].rearrange(
                                    "p (ko j) -> p ko j", ko=2)
                                nc.tensor.matmul(
                                    P[0:H, k2 * TP:(k2 + 1) * TP],
                                    lhsT, rhs,
                                    start=(w == 0), stop=(w == 1),
                                    perf_mode=DR,
                                )
                    if "s_mm" in flags:
                        if pg % 2 == 0:
                            S4 = spool.tile([128, TP], F32, tag="S4",
                                            name=f"S4{pg}")
                        nc.tensor.matmul(
                            S4[64 * (pg % 2):64 * (pg % 2) + 64, :],
                            stab_t[:], ohs_t[:, pg * TP:(pg + 1) * TP],
                            start=True, stop=True,
                            tile_position=(0, 64 * (pg % 2)),
                        )
                    if "epi" in flags:
                        col = (pg % 4) * 2 * TP
                        nc.vector.tensor_tensor(
                            sbe[:, col:col + 2 * TP], P[0:H, :], rst[:], mult)
                        if pg % 2 == 1:
                            q = pg // 2
                            nc.scalar.copy(
                                sbs[:, (q % 4) * TP:(q % 4 + 1) * TP], S4[:])
                    if "dma" in flags and pg % 4 == 3:
                        b = pg // 4
                        hw = 4 * TP
                        nc.sync.dma_start(
                            oedge[:, b * 8 * TP:b * 8 * TP + hw],
                            sbe[:, 0:hw])
                        nc.sync.dma_start(
                            oedge[:, b * 8 * TP + hw:(b + 1) * 8 * TP],
                            sbe[:, hw:2 * hw])
                    if "dma" in flags and pg % 8 == 7:
                        b2 = pg // 8
                        nc.sync.dma_start(
                            ospd[:, b2 * 4 * TP:(b2 + 1) * 4 * TP], sbs[:])
                    if "lo" in flags and pg % 16 == 15:
                        lt2 = pg // 16      # 0 or 1 -> lo tiles 2lt2, 2lt2+1
                        if lt2 == 0:
                            sblo = slopool.tile([H, LOSLOTS], BF16,
                                                tag="sblo", name=f"sblo{pg}")
                        Plo = ppool.tile([128, 2 * TP], F32, tag="P",
                                         name=f"Plo{lt2}")
                        for k2 in range(2):
                            lt = 2 * lt2 + k2
                            for w in range(2):
                                lhsT = wlo_t[:, w * 64:w * 64 + 64].rearrange(
                                    "p (ko h) -> p ko h", ko=2)
                                rb = lt * 2048 + w * 1024
                                rhs = ohlo_t[:, rb:rb + 1024].rearrange(
                                    "p (ko j) -> p ko j", ko=2)
                                nc.tensor.matmul(
                                    Plo[0:H, k2 * TP:(k2 + 1) * TP],
                                    lhsT, rhs,
                                    start=(w == 0), stop=(w == 1),
                                    perf_mode=DR)
                        nc.scalar.copy(
                            sblo[:, lt2 * 2 * TP:(lt2 + 1) * 2 * TP],
                            Plo[0:H, :])
                        if lt2 == 1:
                            nc.sync.dma_start(olo[:], sblo[:])
    nc.compile()
    return nc


def _host_prep(spatial_pos, edge_input, max_dist, spd_table, edge_table,
               edge_dis_weight):
    md = int(max_dist)
    assert md == D
    W = edge_dis_weight.reshape(-1, H, H)[:md].astype(np.float64)
    M = edge_table.astype(np.float64) @ W
    Mflat = M.reshape(512, H)
    Mhi = Mflat.astype(NP8)
    Mlo = (Mflat - Mhi.astype(np.float64)).astype(NP8)

    def pack_w(Ml):
        w = np.zeros((128, 128), NP8)
        for mm in range(2):
            for ko in range(2):
                c0 = mm * 256 + ko * 128
                w[:, mm * 64 + ko * 32:mm * 64 + ko * 32 + 32] = \
                    Ml[c0:c0 + 128]
        return w
    wts = pack_w(Mhi)
    wlo = pack_w(Mlo)

    stab = np.zeros((128, 64), NBF16)
    stab[0:64, 0:32] = spd_table.astype(NBF16)
    stab[64:128, 32:64] = spd_table.astype(NBF16)

    cc = np.arange(512)
    dd = (cc >> 5).astype(np.int32)
    bb = (cc & 31).astype(np.int32)
    svals = np.arange(NS, dtype=np.int32)

    def edge_onehot(e):
        T = e.shape[0]
        oh = (e[:, :, dd] == bb)
        oh2 = oh.transpose(2, 0, 1).reshape(2, 2, 128, T, TP)
        return np.ascontiguousarray(
            oh2.transpose(2, 3, 0, 1, 4).reshape(128, T * 2048)).astype(NP8)

    in_maps = []
    for c in range(NCORES):
        rows = slice(RC * c, RC * (c + 1))
        e = edge_input[rows].reshape(TILES, TP, D)
        ohe = edge_onehot(e)
        sp = spatial_pos[rows]
        spf = sp.reshape(PC)
        oh1 = (spf[None, :] == svals[:, None])
        oh1 = oh1.reshape(NS, PGS, 2, TP)
        ohs = np.zeros((2, NS, PGS, TP), np.bool_)
        ohs[0] = oh1[:, :, 0, :]
        ohs[1] = oh1[:, :, 1, :]
        ohs = np.ascontiguousarray(ohs.reshape(128, PC // 2)).astype(NP8)
        rdev = np.ascontiguousarray(
            (1.0 / np.maximum(spf, 1)).astype(NBF16)[None, :])
        mask = (spf <= 2)
        idxs = np.nonzero(mask)[0]
        assert len(idxs) <= LOSLOTS, f"lo overflow: {len(idxs)}"
        elo = np.zeros((LOSLOTS, D), np.int32)
        elo[:len(idxs)] = e.reshape(PC, D)[idxs]
        ohlo = edge_onehot(elo.reshape(NLO, TP, D))
        in_maps.append({
            "ohe": ohe, "ohlo": ohlo, "ohs": ohs, "wts": wts, "wlo": wlo,
            "stab": stab, "rdev": rdev,
        })
    return in_maps


def _lo_idxs(spatial_pos, c):
    rows = slice(RC * c, RC * (c + 1))
    spf = spatial_pos[rows].reshape(PC)
    return np.nonzero(spf <= 2)[0]


def _host_assemble(results, spatial_pos):
    phi_spd = np.empty((H, N, N), np.float32)
    phi_edge = np.empty((H, N, N), np.float32)
    for c in range(NCORES):
        rs = slice(RC * c, RC * (c + 1))
        a = np.asarray(results[c]["ospd"]).astype(np.float32)
        # [32k+h, q*512+j], tile = 4*q + k (k = (pg%2)*2 + parity)
        a = a.reshape(4, H, TILES // 4, TP).transpose(1, 2, 0, 3)
        phi_spd[:, rs, :] = a.reshape(H, RC, N)
        b = np.asarray(results[c]["oedge"]).astype(np.float32)
        phi_edge[:, rs, :] = b.reshape(H, RC, N)
        idxs = _lo_idxs(spatial_pos, c)
        if len(idxs):
            lo = np.asarray(results[c]["olo"]).astype(np.float32)
            spf = spatial_pos[rs].reshape(PC).astype(np.float32)
            r = 1.0 / np.maximum(spf[idxs], 1)
            flat = phi_edge[:, rs, :].reshape(H, PC)
            flat[:, idxs] += lo[:, :len(idxs)] * r[None, :]
            phi_edge[:, rs, :] = flat.reshape(H, RC, N)
    return phi_spd, phi_edge


def kernel(spatial_pos, edge_input, max_dist, spd_table, edge_table,
           edge_dis_weight, _trace=False):
    spatial_pos = np.asarray(spatial_pos)
    edge_input = np.asarray(edge_input)
    spd_table = np.asarray(spd_table, dtype=np.float32)
    edge_table = np.asarray(edge_table, dtype=np.float32)
    edge_dis_weight = np.asarray(edge_dis_weight, dtype=np.float32)

    if "nc" not in _cached:
        _cached["nc"] = _build_nc()
    nc = _cached["nc"]

    in_maps = _host_prep(spatial_pos, edge_input, max_dist, spd_table,
                         edge_table, edge_dis_weight)
    res = run_bass_kernel_spmd(
        nc, in_maps, core_ids=list(range(NCORES)), trace=bool(_trace)
    )
    out = _host_assemble(res.results, spatial_pos)
    if _trace:
        return out, res
    return out
